# revision 23
# baseline (speedup 1.0000x reference)
"""Distributed multi-head attention kernel for 8 TRN2 NeuronCores (v2).

Problem: x(4,2048,1024) -> qkv proj (w_qkv 3072x1024) -> 16-head attention
(head_dim 64, softmax) -> out proj (w_out 1024x1024 + b_out).

Sharding: head-parallel. Core c owns heads {2c, 2c+1}. After attention a
per-half-batch AllToAll (8 x [8,128,128] bf16) converts head-sharded
attention output to token-sharded layout for the output projection.

v2 design (driven by trace analysis of v1 @ 498us):
 - The PE p-state ramps to 2.4 GHz only after ~3us of gap-free execution
   (1.2 GHz otherwise), so the whole kernel is emitted as ONE continuous
   PE stream: the attention "spine" (S-matmul pairs that row-pack into
   PE row groups 0-63/64-127, 1024-wide exp on ScalarE, PV pipelined two
   groups behind) with QKV-projection-for-next-batch and output-projection
   matmuls woven between spine groups as "filler", sized so the PE never
   waits for ScalarE's exp (PE ~273us of columns vs ScalarE ~274us of exp
   are nearly equal; the schedule keeps both saturated).
 - DMAs are consolidated (one per x token-tile [128,4096], one gather per
   outproj half) so the single sync-queue never head-of-line blocks.
 - A2A is split per half-batch and issued mid-spine right after the half's
   finish; the matching outproj is scheduled as filler ~20 groups later so
   its gather DMA never waits on the collective.
 - The softmax normalize chain has no DMAs: the denominator row (65th V
   column trick) is partition-broadcast raw (GpSimd) and divided on DVE.

Measured v1 baseline: 498us. v2 target ~300-340us.
"""

import numpy as np
import ml_dtypes

import concourse.bass as bass
import concourse.mybir as mybir
import concourse.tile as tile
from concourse import bacc, bass_utils
from concourse.tile import add_dep_helper

FP32 = mybir.dt.float32
BF16 = mybir.dt.bfloat16
AF = mybir.ActivationFunctionType
DIV = mybir.AluOpType.divide

N_CORES = 8
B, NTOK, D = 4, 2048, 1024
T = B * NTOK  # 8192 tokens total
NH, HD = 16, 64
HL = NH // N_CORES  # 2 heads per core
SCALE = float(HD) ** -0.5  # 0.125
TN = 512  # token tile for QKV / q tile for attention
KC = D // 128  # 8 contraction chunks for projections
KT = NTOK // 128  # 16 k-chunks per batch in attention
WCOLS = 3 * HL * HD  # 384 qkv output dims per core


def build_nc(debug=False):
    nc = bacc.Bacc(
        "TRN2", target_bir_lowering=False, debug=False, num_devices=N_CORES
    )
    xt = nc.dram_tensor("xt", [D, T], BF16, kind="ExternalInput").ap()
    wt = nc.dram_tensor("wt", [D, WCOLS], BF16, kind="ExternalInput").ap()
    wo = nc.dram_tensor("wo", [D, D], BF16, kind="ExternalInput").ap()
    bias = nc.dram_tensor("bias", [1, D], FP32, kind="ExternalInput").ap()
    # row r of out = batch r//256, half (r//128)%2, token (core*128 + r%128)
    out = nc.dram_tensor("out", [B * 2 * 128, D], FP32, kind="ExternalOutput").ap()
    if debug:
        qdump = nc.dram_tensor("qdump", [128, T], FP32, kind="ExternalOutput").ap()
        kdump = nc.dram_tensor("kdump", [128, T], FP32, kind="ExternalOutput").ap()
        vdump = nc.dram_tensor(
            "vdump", [128, (T // 128) * HL * 65], FP32, kind="ExternalOutput"
        ).ap()
        ocdump = nc.dram_tensor("ocdump", [65, 512], FP32, kind="ExternalOutput").ap()
        bcdump = nc.dram_tensor("bcdump", [64, 512], FP32, kind="ExternalOutput").ap()
        rcdump = nc.dram_tensor("rcdump", [64, 512], FP32, kind="ExternalOutput").ap()
        otdump = nc.dram_tensor("otdump", [64, 512], FP32, kind="ExternalOutput").ap()
        adump = nc.dram_tensor(
            "adump", [N_CORES, HL * HD, 128], FP32, kind="ExternalOutput"
        ).ap()

    with tile.TileContext(nc) as tc:
        with (
            tc.tile_pool(name="const", bufs=1) as const,
            tc.tile_pool(name="xin", bufs=6) as xin,
            tc.tile_pool(name="probs", bufs=6) as probs,
            tc.tile_pool(name="norm", bufs=3) as norm,
            tc.tile_pool(name="ot", bufs=3) as otp,
            tc.tile_pool(name="osb", bufs=2) as osbp,
            tc.tile_pool(name="fin", bufs=2) as fin,
            tc.tile_pool(name="psum", bufs=2, space="PSUM") as psum,
            tc.tile_pool(name="dram", bufs=1, space="DRAM") as dram,
        ):
            # ---- persistent SBUF state ----
            w_sb = const.tile([128, KC * WCOLS], BF16)
            q_sb = const.tile([128, T], BF16)  # [2 heads x 64, tokens] scaled
            k_sb = const.tile([128, T], BF16)
            # V token-major: [128 tok-in-chunk, (global chunk, head) x 65]
            # 65 cols per (chunk, head): [ones | 64 V dims] -- ones FIRST so
            # the PV denominator row lands on partition 0 (partition_broadcast
            # reads partition 0 of its input AP regardless of offset).
            v_sb = const.tile([128, (T // 128) * HL * 65], BF16)
            v3 = v_sb[:].rearrange("p (blk e) -> p blk e", e=65)
            nc.vector.memset(v3[:, :, 0:1], 1.0)

            x_tiles = {}

            def load_x_tile(t):
                x_t = xin.tile([128, KC * TN], BF16, tag="xt", name="x_t")
                nc.sync.dma_start(
                    x_t[:].rearrange("p (kc e) -> p kc e", kc=KC),
                    xt[:, t * TN : (t + 1) * TN].rearrange(
                        "(kc p) e -> p kc e", p=128
                    ),
                )
                x_tiles[t] = x_t

            # startup: interleave per-chunk w and x(t0) loads so the first
            # QKV matmul can start ~9us in and pipeline behind the DMAs
            x_t0 = xin.tile([128, KC * TN], BF16, tag="xt", name="x_t0")
            x_tiles[0] = x_t0
            for kc in range(KC):
                nc.sync.dma_start(
                    w_sb[:, kc * WCOLS : (kc + 1) * WCOLS],
                    wt[kc * 128 : (kc + 1) * 128, :],
                )
                nc.sync.dma_start(
                    x_t0[:, kc * TN : (kc + 1) * TN],
                    xt[kc * 128 : (kc + 1) * 128, 0:TN],
                )
            # tiny prologue collective: absorbs inter-core launch skew while
            # the PE is busy with QKV(0), so the first real A2A is not ~20us
            sk_in = dram.tile([N_CORES, 1, 64], BF16, name="sk_in")
            sk_out = dram.tile([N_CORES, 1, 64], BF16, name="sk_out")
            nc.gpsimd.collective_compute(
                "AllToAll",
                mybir.AluOpType.bypass,
                replica_groups=[list(range(N_CORES))],
                ins=[sk_in.opt()],
                outs=[sk_out.opt()],
            )
            load_x_tile(1)

            b_row = const.tile([1, D], FP32)
            nc.sync.dma_start(b_row[:], bias[:])
            bias_sb = const.tile([128, D], FP32)
            nc.gpsimd.partition_broadcast(bias_sb[:], b_row[:])

            load_x_tile(2)
            load_x_tile(3)

            wo_sb = const.tile([128, KC * D], BF16)
            for half in range(2):
                nc.sync.dma_start(
                    wo_sb[:, half * 4 * D : (half + 1) * 4 * D].rearrange(
                        "p (kc n) -> p kc n", kc=4
                    ),
                    wo[half * 512 : (half + 1) * 512, :].rearrange(
                        "(kc p) n -> p kc n", p=128
                    ),
                )

            a2a_in = {}
            a2a_out = {}
            for b in range(B):
                for hf in range(2):
                    a2a_in[(b, hf)] = dram.tile(
                        [N_CORES, HL * HD, 128], BF16, name=f"a2a_in{b}_{hf}"
                    )
                    a2a_out[(b, hf)] = dram.tile(
                        [N_CORES, HL * HD, 128], BF16, name=f"a2a_out{b}_{hf}"
                    )

            def emit_a2a(b, hf):
                nc.gpsimd.collective_compute(
                    "AllToAll",
                    mybir.AluOpType.bypass,
                    replica_groups=[list(range(N_CORES))],
                    ins=[a2a_in[(b, hf)].opt()],
                    outs=[a2a_out[(b, hf)].opt()],
                )

            # ---- filler generators (yield approx emitted PE columns) ----
            def gen_qkv_sec(t, which):
                """One section (K, V, or Q) of the QKV projection for global
                token-tile t. Emitted as filler between spine groups."""
                if t not in x_tiles:  # fallback; normally prefetched
                    load_x_tile(t)
                x_t = x_tiles[t]
                fil = psum.tile([128, 512], FP32, tag="fil", name=f"fil_{which}")
                if which == "v":
                    prev = None
                    for s in range(4):
                        for kc in range(KC):
                            mm = nc.tensor.matmul(
                                fil[:, s * 128 : (s + 1) * 128],
                                lhsT=x_t[
                                    :, kc * TN + s * 128 : kc * TN + (s + 1) * 128
                                ],
                                rhs=w_sb[:, kc * WCOLS + 256 : kc * WCOLS + WCOLS],
                                start=(kc == 0),
                                stop=(kc == KC - 1),
                            )
                            if prev is not None:
                                add_dep_helper(
                                    mm.ins, prev.ins, sync=False,
                                    reason="bank flag-clear order",
                                )
                            prev = mm
                            if kc % 2 == 1:
                                yield 256
                    nc.vector.tensor_copy(
                        v3[:, (t * 4) * HL : (t * 4 + 4) * HL, 1:65],
                        fil[:]
                        .rearrange("p (s hd) -> p s hd", s=4)
                        .rearrange("p s (h d) -> p (s h) d", h=HL),
                    )
                else:
                    m = 0 if which == "q" else 1
                    for kc in range(KC):
                        nc.tensor.matmul(
                            fil[:, :],
                            lhsT=w_sb[
                                :,
                                kc * WCOLS + m * 128 : kc * WCOLS + (m + 1) * 128,
                            ],
                            rhs=x_t[:, kc * TN : (kc + 1) * TN],
                            start=(kc == 0),
                            stop=(kc == KC - 1),
                        )
                        yield 512
                    if which == "q":
                        nc.vector.tensor_scalar_mul(
                            q_sb[:, t * TN : (t + 1) * TN], fil[:], SCALE
                        )
                    else:
                        nc.vector.tensor_copy(
                            k_sb[:, t * TN : (t + 1) * TN], fil[:]
                        )

            def gen_outproj(b, hf):
                """Output projection for half-batch (b, hf): 128 tokens."""
                o_sb = osbp.tile([128, N_CORES * 128], BF16, tag="osb", name="o_sb")
                nc.sync.dma_start(
                    o_sb[:].rearrange("p (i e) -> p i e", i=N_CORES),
                    a2a_out[(b, hf)][:, :, :].rearrange("i p e -> p i e"),
                )
                yield 0
                o_ps = [
                    psum.tile([128, 512], FP32, tag="fil", name=f"o_ps{nh}")
                    for nh in range(2)
                ]
                for i in range(N_CORES):
                    for nh in range(2):
                        nc.tensor.matmul(
                            o_ps[nh][:, :],
                            lhsT=o_sb[:, i * 128 : (i + 1) * 128],
                            rhs=wo_sb[:, i * D + nh * 512 : i * D + nh * 512 + 512],
                            start=(i == 0),
                            stop=(i == N_CORES - 1),
                        )
                        yield 512
                out_t = fin.tile([128, D], FP32, tag="outt", name="out_t")
                for nh in range(2):
                    nc.vector.tensor_add(
                        out_t[:, nh * 512 : (nh + 1) * 512],
                        o_ps[nh][:, :],
                        bias_sb[:, nh * 512 : (nh + 1) * 512],
                    )
                nc.sync.dma_start(
                    out[(b * 2 + hf) * 128 : (b * 2 + hf + 1) * 128, :], out_t[:]
                )

            fillers = []  # deque of generators, pulled from the front
            pump_bal = [0.0]  # cumulative col budget (no overshoot drift)

            def pump(quota):
                pump_bal[0] += quota
                while pump_bal[0] > 0 and fillers:
                    try:
                        pump_bal[0] -= next(fillers[0])
                    except StopIteration:
                        fillers.pop(0)
                if not fillers:
                    pump_bal[0] = 0.0

            def exhaust(n):
                """Run the first n generators (or all if n<0) to completion."""
                cnt = 0
                while fillers and (n < 0 or cnt < n):
                    g = fillers.pop(0)
                    for _ in g:
                        pass
                    cnt += 1

            # ---- prologue: QKV for batch-0 tile 0, emitted inline ----
            for sec in ("k", "v", "q"):
                for _ in gen_qkv_sec(0, sec):
                    pass

            # ---- spine: attention, with QKV(b+1)/outproj woven in ----
            pending = []  # (gc, p_t, pv) with S+exp emitted, PV not yet
            fin_q = []  # [(b, qt, pv)] awaiting normalize+a2a_in write

            def flush_one():
                gc, p_t, pv = pending.pop(0)
                for h in range(HL):
                    nc.tensor.matmul(
                        pv[h][0:65, :],
                        lhsT=v3[:, gc * HL + h, :],
                        rhs=p_t[:, h * 512 : (h + 1) * 512],
                        start=(gc % KT == 0),
                        stop=(gc % KT == KT - 1),
                    )

            def finish_qt(b, qt, pv):
                for h in range(HL):
                    # pv row 0 = softmax denominator (ones-first V layout)
                    o_c = norm.tile([65, 512], FP32, tag="oc", name="o_c")
                    nc.vector.tensor_copy(o_c[:], pv[h][0:65, :])
                    # reciprocal is ~6 cyc/elem per lane: DMA-reshape the 512
                    # denominators across 128 partitions so it runs 4/lane
                    rs = norm.tile([128, 4], FP32, tag="rs", name="rs")
                    nc.sync.dma_start(rs[:], o_c[0:1, :])
                    rr = norm.tile([128, 4], FP32, tag="rr", name="rr")
                    nc.vector.reciprocal(rr[:], rs[:])
                    rec = norm.tile([1, 512], FP32, tag="rec", name="rec")
                    nc.sync.dma_start(rec[:], rr[:])
                    bc = norm.tile([65, 512], FP32, tag="bc", name="bc")
                    nc.gpsimd.partition_broadcast(bc[:], rec[:])
                    # row 0 = denom * (1/denom) = 1; rows 1:65 = normalized O
                    o_t = otp.tile([65, 512], BF16, tag="o", name="o_t")
                    nc.vector.tensor_mul(o_t[:], o_c[:], bc[:])
                    if debug and b == 0 and qt == 0 and h == 0:
                        nc.sync.dma_start(ocdump[:, :], o_c[:])
                        nc.sync.dma_start(bcdump[:, :], bc[0:64, :])
                        nc.sync.dma_start(rcdump[:, :], bc[1:65, :])
                        ot32 = norm.tile([65, 512], FP32, tag="od", name="ot32")
                        nc.vector.tensor_copy(ot32[:], o_t[:])
                        nc.sync.dma_start(otdump[:, :], ot32[1:65, :])
                    j0 = (qt % 2) * 4
                    nc.sync.dma_start(
                        a2a_in[(b, qt // 2)][
                            j0 : j0 + 4, h * 64 : (h + 1) * 64, :
                        ].rearrange("j p e -> p j e"),
                        o_t[1:65, :].rearrange("p (j e) -> p j e", j=4),
                    )
                if qt % 2 == 1:
                    emit_a2a(b, qt // 2)

            for b in range(B):
                # filler inventory for this batch's spine
                if b == 0:
                    for t in (1, 2, 3):
                        fillers.append(gen_qkv_sec(t, "k"))
                        fillers.append(gen_qkv_sec(t, "v"))
                    for t in (1, 2, 3):
                        fillers.append(gen_qkv_sec(t, "q"))
                    for t in range(4, 8):
                        for sec in ("k", "v", "q"):
                            fillers.append(gen_qkv_sec(t, sec))
                    # outproj(0,0) deferred to b1: the first A2A is slow
                elif b == 1:
                    for sec in ("k", "v", "q"):
                        fillers.append(gen_qkv_sec(8, sec))
                    fillers.append(gen_outproj(0, 0))
                    fillers.append(gen_outproj(0, 1))
                    for sec in ("k", "v", "q"):
                        fillers.append(gen_qkv_sec(9, sec))
                    for sec in ("k", "v", "q"):
                        fillers.append(gen_qkv_sec(10, sec))
                    fillers.append(gen_outproj(1, 0))
                    for sec in ("k", "v", "q"):
                        fillers.append(gen_qkv_sec(11, sec))
                elif b == 2:
                    for sec in ("k", "v", "q"):
                        fillers.append(gen_qkv_sec(12, sec))
                    fillers.append(gen_outproj(1, 1))
                    for sec in ("k", "v", "q"):
                        fillers.append(gen_qkv_sec(13, sec))
                    for sec in ("k", "v", "q"):
                        fillers.append(gen_qkv_sec(14, sec))
                    fillers.append(gen_outproj(2, 0))
                    for sec in ("k", "v", "q"):
                        fillers.append(gen_qkv_sec(15, sec))
                else:
                    fillers.append(gen_outproj(b - 1, 1))
                    # gen_outproj(b, 0) is appended mid-spine, after its A2A

                if b == 0:
                    quotas = (2304, 1024, 1024, 1024)
                elif b == 1:
                    quotas = (1152, 1152, 1152, 1152)
                elif b == 2:
                    quotas = (1024, 1024, 1024, 1024)
                else:
                    quotas = (320, 320, 320, 320)
                for qt in range(4):
                    quota0 = quotas[qt]
                    # prefetch next batch's x tiles (consolidated DMAs)
                    if b < B - 1:
                        nt = 4 * (b + 1)
                        if qt == 0:
                            load_x_tile(nt + 0)
                            load_x_tile(nt + 1)
                        elif qt == 1:
                            load_x_tile(nt + 2)
                            load_x_tile(nt + 3)
                    pv = [
                        psum.tile([128, 512], FP32, tag="pv", name=f"pv{h}")
                        for h in range(HL)
                    ]
                    q_off = b * NTOK + qt * TN
                    for kc in range(KT):
                        s_t = psum.tile([128, 1024], FP32, tag="st", name="s_t")
                        for h in range(HL):
                            nc.tensor.matmul(
                                s_t[:, h * 512 : (h + 1) * 512],
                                lhsT=k_sb[
                                    h * 64 : (h + 1) * 64,
                                    b * NTOK + kc * 128 : b * NTOK + (kc + 1) * 128,
                                ],
                                rhs=q_sb[h * 64 : (h + 1) * 64, q_off : q_off + TN],
                                start=True,
                                stop=True,
                            )
                        p_t = probs.tile([128, 1024], BF16, tag="p", name="p_t")
                        nc.scalar.activation(p_t[:], s_t[:], AF.Exp)
                        pending.append((b * KT + kc, p_t, pv))
                        if kc == 2 and fin_q:
                            finish_qt(*fin_q.pop(0))
                            if b == B - 1 and qt == 2:
                                # A2A(3,0) just emitted; safe to schedule its
                                # outproj as filler from here on
                                fillers.append(gen_outproj(b, 0))
                        # hold b3's filler until its A2A deps are in flight
                        if not (b == B - 1 and qt == 0 and kc < 3):
                            pump(quota0)
                        # shorten the PV pipeline at the very end so the
                        # final finish/A2A trigger as early as possible
                        lag = 1 if (b == B - 1 and qt == 3 and kc >= 12) else 2
                        while len(pending) > lag:
                            flush_one()
                    fin_q.append((b, qt, pv))

            # ---- tail ----
            while pending:
                flush_one()
            finish_qt(*fin_q.pop(0))  # qt3 of batch 3 -> emits A2A(3,1)
            exhaust(-1)  # leftover filler (outproj(3,0) tail) overlaps the A2A
            # p-state keepalive: harmless matmuls keep the PE clocked up
            # while the final A2A + gather complete, so the last outproj
            # runs at full rate instead of the cold ~2x-slow rate
            dum = psum.tile([128, 512], FP32, tag="fil", name="dum")
            for _ in range(56):
                nc.tensor.matmul(
                    dum[:, :], lhsT=w_sb[:, 0:128], rhs=wo_sb[:, 0:512],
                    start=True, stop=True,
                )
            for _ in gen_outproj(B - 1, 1):
                pass

            if debug:
                for t in range(T // TN):
                    d1 = fin.tile([128, TN], FP32, tag="outt", name="d1")
                    nc.vector.tensor_copy(d1[:], q_sb[:, t * TN : (t + 1) * TN])
                    nc.sync.dma_start(qdump[:, t * TN : (t + 1) * TN], d1[:])
                    d2 = fin.tile([128, TN], FP32, tag="outt", name="d2")
                    nc.vector.tensor_copy(d2[:], k_sb[:, t * TN : (t + 1) * TN])
                    nc.sync.dma_start(kdump[:, t * TN : (t + 1) * TN], d2[:])
                nv = (T // 128) * HL * 65
                for j in range(0, nv, 1024):
                    wdt = min(1024, nv - j)
                    d3 = fin.tile([128, 1024], FP32, tag="outt", name="d3")
                    nc.vector.tensor_copy(d3[:, 0:wdt], v_sb[:, j : j + wdt])
                    nc.sync.dma_start(vdump[:, j : j + wdt], d3[:, 0:wdt])
                for i in range(N_CORES):
                    d4 = fin.tile([128, 128], BF16, tag="d4", name="d4")
                    nc.sync.dma_start(d4[:], a2a_in[(0, 0)][i, :, :])
                    d5 = fin.tile([128, 128], FP32, tag="outt", name="d5")
                    nc.vector.tensor_copy(d5[:], d4[:])
                    nc.sync.dma_start(adump[i, :, :], d5[:])

    nc.compile()
    return nc


_NC_CACHE = None


def _get_nc():
    global _NC_CACHE
    if _NC_CACHE is None:
        _NC_CACHE = build_nc()
    return _NC_CACHE


def make_in_maps(x, w_qkv, w_out, b_out):
    x = np.asarray(x, dtype=np.float32)
    w_qkv = np.asarray(w_qkv, dtype=np.float32)
    w_out = np.asarray(w_out, dtype=np.float32)
    b_out = np.asarray(b_out, dtype=np.float32)

    xt_np = np.ascontiguousarray(x.reshape(T, D).T).astype(ml_dtypes.bfloat16)
    wo_np = np.ascontiguousarray(w_out.T).astype(ml_dtypes.bfloat16)
    b_np = np.ascontiguousarray(b_out.reshape(1, D))

    in_maps = []
    for c in range(N_CORES):
        rows = []
        for sec in range(3):  # q, k, v sections of w_qkv
            for hh in range(HL):
                h = HL * c + hh
                rows.append(w_qkv[sec * D + h * HD : sec * D + (h + 1) * HD, :])
        wt_np = np.ascontiguousarray(np.concatenate(rows, 0).T).astype(
            ml_dtypes.bfloat16
        )  # (1024, 384)
        in_maps.append({"xt": xt_np, "wt": wt_np, "wo": wo_np, "bias": b_np})
    return in_maps


def kernel(x, w_qkv, w_out, b_out, _trace=False, _tmpdir=None):
    in_maps = make_in_maps(x, w_qkv, w_out, b_out)
    nc = _get_nc()
    res = bass_utils.run_bass_kernel_spmd(
        nc, in_maps, core_ids=list(range(N_CORES)), trace=_trace, tmpdir=_tmpdir
    )
    # core j out rows: (b, hf) block r = (2b+hf)*128 + u
    #   -> token b*2048 + hf*1024 + j*128 + u of the full output
    full = np.empty((T, D), np.float32)
    for j in range(N_CORES):
        o = np.asarray(res.results[j]["out"], dtype=np.float32)
        for b in range(B):
            for hf in range(2):
                dst = b * NTOK + hf * 1024 + j * 128
                src = (b * 2 + hf) * 128
                full[dst : dst + 128] = o[src : src + 128]
    kernel.last_result = res
    return full.reshape(B, NTOK, D)


# revision 24
# speedup vs baseline: 1.0481x; 1.0481x over previous
"""Distributed multi-head attention kernel for 8 TRN2 NeuronCores (v2).

Problem: x(4,2048,1024) -> qkv proj (w_qkv 3072x1024) -> 16-head attention
(head_dim 64, softmax) -> out proj (w_out 1024x1024 + b_out).

Sharding: head-parallel. Core c owns heads {2c, 2c+1}. After attention a
per-half-batch AllToAll (8 x [8,128,128] bf16) converts head-sharded
attention output to token-sharded layout for the output projection.

v2 design (driven by trace analysis of v1 @ 498us):
 - The PE p-state ramps to 2.4 GHz only after ~3us of gap-free execution
   (1.2 GHz otherwise), so the whole kernel is emitted as ONE continuous
   PE stream: the attention "spine" (S-matmul pairs that row-pack into
   PE row groups 0-63/64-127, 1024-wide exp on ScalarE, PV pipelined two
   groups behind) with QKV-projection-for-next-batch and output-projection
   matmuls woven between spine groups as "filler", sized so the PE never
   waits for ScalarE's exp (PE ~273us of columns vs ScalarE ~274us of exp
   are nearly equal; the schedule keeps both saturated).
 - DMAs are consolidated (one per x token-tile [128,4096], one gather per
   outproj half) so the single sync-queue never head-of-line blocks.
 - A2A is split per half-batch and issued mid-spine right after the half's
   finish; the matching outproj is scheduled as filler ~20 groups later so
   its gather DMA never waits on the collective.
 - The softmax normalize chain has no DMAs: the denominator row (65th V
   column trick) is partition-broadcast raw (GpSimd) and divided on DVE.

Measured v1 baseline: 498us. v2 target ~300-340us.
"""

import numpy as np
import ml_dtypes

import concourse.bass as bass
import concourse.mybir as mybir
import concourse.tile as tile
from concourse import bacc, bass_utils
from concourse.tile import add_dep_helper

FP32 = mybir.dt.float32
BF16 = mybir.dt.bfloat16
AF = mybir.ActivationFunctionType
DIV = mybir.AluOpType.divide

N_CORES = 8
B, NTOK, D = 4, 2048, 1024
T = B * NTOK  # 8192 tokens total
NH, HD = 16, 64
HL = NH // N_CORES  # 2 heads per core
SCALE = float(HD) ** -0.5  # 0.125
TN = 512  # token tile for QKV / q tile for attention
KC = D // 128  # 8 contraction chunks for projections
KT = NTOK // 128  # 16 k-chunks per batch in attention
WCOLS = 3 * HL * HD  # 384 qkv output dims per core


def build_nc(debug=False):
    nc = bacc.Bacc(
        "TRN2", target_bir_lowering=False, debug=False, num_devices=N_CORES
    )
    xt = nc.dram_tensor("xt", [D, T], BF16, kind="ExternalInput").ap()
    wt = nc.dram_tensor("wt", [D, WCOLS], BF16, kind="ExternalInput").ap()
    wo = nc.dram_tensor("wo", [D, D], BF16, kind="ExternalInput").ap()
    bias = nc.dram_tensor("bias", [1, D], FP32, kind="ExternalInput").ap()
    # row r of out = batch r//256, half (r//128)%2, token (core*128 + r%128)
    out = nc.dram_tensor("out", [B * 2 * 128, D], FP32, kind="ExternalOutput").ap()
    if debug:
        qdump = nc.dram_tensor("qdump", [128, T], FP32, kind="ExternalOutput").ap()
        kdump = nc.dram_tensor("kdump", [128, T], FP32, kind="ExternalOutput").ap()
        vdump = nc.dram_tensor(
            "vdump", [128, (T // 128) * HL * 65], FP32, kind="ExternalOutput"
        ).ap()
        ocdump = nc.dram_tensor("ocdump", [65, 512], FP32, kind="ExternalOutput").ap()
        bcdump = nc.dram_tensor("bcdump", [64, 512], FP32, kind="ExternalOutput").ap()
        rcdump = nc.dram_tensor("rcdump", [64, 512], FP32, kind="ExternalOutput").ap()
        otdump = nc.dram_tensor("otdump", [64, 512], FP32, kind="ExternalOutput").ap()
        adump = nc.dram_tensor(
            "adump", [N_CORES, HL * HD, 128], FP32, kind="ExternalOutput"
        ).ap()

    with tile.TileContext(nc) as tc:
        with (
            tc.tile_pool(name="const", bufs=1) as const,
            tc.tile_pool(name="xin", bufs=6) as xin,
            tc.tile_pool(name="probs", bufs=6) as probs,
            tc.tile_pool(name="norm", bufs=3) as norm,
            tc.tile_pool(name="ot", bufs=3) as otp,
            tc.tile_pool(name="osb", bufs=2) as osbp,
            tc.tile_pool(name="fin", bufs=2) as fin,
            tc.tile_pool(name="psum", bufs=2, space="PSUM") as psum,
            tc.tile_pool(name="dram", bufs=1, space="DRAM") as dram,
        ):
            # ---- persistent SBUF state ----
            w_sb = const.tile([128, KC * WCOLS], BF16)
            q_sb = const.tile([128, T], BF16)  # [2 heads x 64, tokens] scaled
            k_sb = const.tile([128, T], BF16)
            # V token-major: [128 tok-in-chunk, (global chunk, head) x 65]
            # 65 cols per (chunk, head): [ones | 64 V dims] -- ones FIRST so
            # the PV denominator row lands on partition 0 (partition_broadcast
            # reads partition 0 of its input AP regardless of offset).
            v_sb = const.tile([128, (T // 128) * HL * 65], BF16)
            v3 = v_sb[:].rearrange("p (blk e) -> p blk e", e=65)
            nc.vector.memset(v3[:, :, 0:1], 1.0)

            x_tiles = {}

            def load_x_tile(t):
                x_t = xin.tile([128, KC * TN], BF16, tag="xt", name="x_t")
                nc.sync.dma_start(
                    x_t[:].rearrange("p (kc e) -> p kc e", kc=KC),
                    xt[:, t * TN : (t + 1) * TN].rearrange(
                        "(kc p) e -> p kc e", p=128
                    ),
                )
                x_tiles[t] = x_t

            # startup: interleave per-chunk w and x(t0) loads so the first
            # QKV matmul can start ~9us in and pipeline behind the DMAs
            x_t0 = xin.tile([128, KC * TN], BF16, tag="xt", name="x_t0")
            x_tiles[0] = x_t0
            for kc in range(KC):
                nc.sync.dma_start(
                    w_sb[:, kc * WCOLS : (kc + 1) * WCOLS],
                    wt[kc * 128 : (kc + 1) * 128, :],
                )
                nc.sync.dma_start(
                    x_t0[:, kc * TN : (kc + 1) * TN],
                    xt[kc * 128 : (kc + 1) * 128, 0:TN],
                )
            # tiny prologue collective: absorbs inter-core launch skew while
            # the PE is busy with QKV(0), so the first real A2A is not ~20us
            sk_in = dram.tile([N_CORES, 1, 64], BF16, name="sk_in")
            sk_out = dram.tile([N_CORES, 1, 64], BF16, name="sk_out")
            nc.gpsimd.collective_compute(
                "AllToAll",
                mybir.AluOpType.bypass,
                replica_groups=[list(range(N_CORES))],
                ins=[sk_in.opt()],
                outs=[sk_out.opt()],
            )
            load_x_tile(1)

            b_row = const.tile([1, D], FP32)
            nc.sync.dma_start(b_row[:], bias[:])
            bias_sb = const.tile([128, D], FP32)
            nc.gpsimd.partition_broadcast(bias_sb[:], b_row[:])

            load_x_tile(2)
            load_x_tile(3)

            wo_sb = const.tile([128, KC * D], BF16)
            for half in range(2):
                nc.sync.dma_start(
                    wo_sb[:, half * 4 * D : (half + 1) * 4 * D].rearrange(
                        "p (kc n) -> p kc n", kc=4
                    ),
                    wo[half * 512 : (half + 1) * 512, :].rearrange(
                        "(kc p) n -> p kc n", p=128
                    ),
                )

            a2a_in = {}
            a2a_out = {}
            for b in range(B):
                for hf in range(2):
                    a2a_in[(b, hf)] = dram.tile(
                        [N_CORES, HL * HD, 128], BF16, name=f"a2a_in{b}_{hf}"
                    )
                    a2a_out[(b, hf)] = dram.tile(
                        [N_CORES, HL * HD, 128], BF16, name=f"a2a_out{b}_{hf}"
                    )

            def emit_a2a(b, hf):
                nc.gpsimd.collective_compute(
                    "AllToAll",
                    mybir.AluOpType.bypass,
                    replica_groups=[list(range(N_CORES))],
                    ins=[a2a_in[(b, hf)].opt()],
                    outs=[a2a_out[(b, hf)].opt()],
                )

            # ---- filler generators (yield approx emitted PE columns) ----
            def gen_qkv_sec(t, which):
                """One section (K, V, or Q) of the QKV projection for global
                token-tile t. Emitted as filler between spine groups."""
                if t not in x_tiles:  # fallback; normally prefetched
                    load_x_tile(t)
                x_t = x_tiles[t]
                fil = psum.tile([128, 512], FP32, tag="fil", name=f"fil_{which}")
                if which == "v":
                    prev = None
                    for s in range(4):
                        for kc in range(KC):
                            mm = nc.tensor.matmul(
                                fil[:, s * 128 : (s + 1) * 128],
                                lhsT=x_t[
                                    :, kc * TN + s * 128 : kc * TN + (s + 1) * 128
                                ],
                                rhs=w_sb[:, kc * WCOLS + 256 : kc * WCOLS + WCOLS],
                                start=(kc == 0),
                                stop=(kc == KC - 1),
                            )
                            if prev is not None:
                                add_dep_helper(
                                    mm.ins, prev.ins, sync=False,
                                    reason="bank flag-clear order",
                                )
                            prev = mm
                            if kc % 2 == 1:
                                yield 256
                    nc.vector.tensor_copy(
                        v3[:, (t * 4) * HL : (t * 4 + 4) * HL, 1:65],
                        fil[:]
                        .rearrange("p (s hd) -> p s hd", s=4)
                        .rearrange("p s (h d) -> p (s h) d", h=HL),
                    )
                else:
                    m = 0 if which == "q" else 1
                    for kc in range(KC):
                        nc.tensor.matmul(
                            fil[:, :],
                            lhsT=w_sb[
                                :,
                                kc * WCOLS + m * 128 : kc * WCOLS + (m + 1) * 128,
                            ],
                            rhs=x_t[:, kc * TN : (kc + 1) * TN],
                            start=(kc == 0),
                            stop=(kc == KC - 1),
                        )
                        yield 512
                    if which == "q":
                        nc.vector.tensor_scalar_mul(
                            q_sb[:, t * TN : (t + 1) * TN], fil[:], SCALE
                        )
                    else:
                        nc.vector.tensor_copy(
                            k_sb[:, t * TN : (t + 1) * TN], fil[:]
                        )

            def gen_outproj(b, hf):
                """Output projection for half-batch (b, hf): 128 tokens."""
                o_sb = osbp.tile([128, N_CORES * 128], BF16, tag="osb", name="o_sb")
                nc.sync.dma_start(
                    o_sb[:].rearrange("p (i e) -> p i e", i=N_CORES),
                    a2a_out[(b, hf)][:, :, :].rearrange("i p e -> p i e"),
                )
                yield 0
                o_ps = [
                    psum.tile([128, 512], FP32, tag="fil", name=f"o_ps{nh}")
                    for nh in range(2)
                ]
                for i in range(N_CORES):
                    for nh in range(2):
                        nc.tensor.matmul(
                            o_ps[nh][:, :],
                            lhsT=o_sb[:, i * 128 : (i + 1) * 128],
                            rhs=wo_sb[:, i * D + nh * 512 : i * D + nh * 512 + 512],
                            start=(i == 0),
                            stop=(i == N_CORES - 1),
                        )
                        yield 512
                out_t = fin.tile([128, D], FP32, tag="outt", name="out_t")
                for nh in range(2):
                    nc.vector.tensor_add(
                        out_t[:, nh * 512 : (nh + 1) * 512],
                        o_ps[nh][:, :],
                        bias_sb[:, nh * 512 : (nh + 1) * 512],
                    )
                nc.sync.dma_start(
                    out[(b * 2 + hf) * 128 : (b * 2 + hf + 1) * 128, :], out_t[:]
                )

            fillers = []  # deque of generators, pulled from the front
            pump_bal = [0.0]  # cumulative col budget (no overshoot drift)

            def pump(quota):
                pump_bal[0] += quota
                while pump_bal[0] > 0 and fillers:
                    try:
                        pump_bal[0] -= next(fillers[0])
                    except StopIteration:
                        fillers.pop(0)
                if not fillers:
                    pump_bal[0] = 0.0

            def exhaust(n):
                """Run the first n generators (or all if n<0) to completion."""
                cnt = 0
                while fillers and (n < 0 or cnt < n):
                    g = fillers.pop(0)
                    for _ in g:
                        pass
                    cnt += 1

            # ---- prologue: QKV for batch-0 tile 0, emitted inline ----
            for sec in ("k", "v", "q"):
                for _ in gen_qkv_sec(0, sec):
                    pass

            # ---- spine: attention, with QKV(b+1)/outproj woven in ----
            pending = []  # (gc, p_t, pv) with S+exp emitted, PV not yet
            fin_q = []  # [(b, qt, pv)] awaiting normalize+a2a_in write

            def flush_one():
                gc, p_t, pv = pending.pop(0)
                for h in range(HL):
                    nc.tensor.matmul(
                        pv[h][0:65, :],
                        lhsT=v3[:, gc * HL + h, :],
                        rhs=p_t[:, h * 512 : (h + 1) * 512],
                        start=(gc % KT == 0),
                        stop=(gc % KT == KT - 1),
                    )

            def finish_qt(b, qt, pv):
                for h in range(HL):
                    # pv row 0 = softmax denominator (ones-first V layout)
                    o_c = norm.tile([65, 512], FP32, tag="oc", name="o_c")
                    nc.vector.tensor_copy(o_c[:], pv[h][0:65, :])
                    # reciprocal is ~6 cyc/elem per lane: DMA-reshape the 512
                    # denominators across 128 partitions so it runs 4/lane
                    rs = norm.tile([128, 4], FP32, tag="rs", name="rs")
                    nc.sync.dma_start(rs[:], o_c[0:1, :])
                    rr = norm.tile([128, 4], FP32, tag="rr", name="rr")
                    nc.vector.reciprocal(rr[:], rs[:])
                    rec = norm.tile([1, 512], FP32, tag="rec", name="rec")
                    nc.sync.dma_start(rec[:], rr[:])
                    bc = norm.tile([65, 512], FP32, tag="bc", name="bc")
                    nc.gpsimd.partition_broadcast(bc[:], rec[:])
                    # row 0 = denom * (1/denom) = 1; rows 1:65 = normalized O
                    o_t = otp.tile([65, 512], BF16, tag="o", name="o_t")
                    nc.vector.tensor_mul(o_t[:], o_c[:], bc[:])
                    if debug and b == 0 and qt == 0 and h == 0:
                        nc.sync.dma_start(ocdump[:, :], o_c[:])
                        nc.sync.dma_start(bcdump[:, :], bc[0:64, :])
                        nc.sync.dma_start(rcdump[:, :], bc[1:65, :])
                        ot32 = norm.tile([65, 512], FP32, tag="od", name="ot32")
                        nc.vector.tensor_copy(ot32[:], o_t[:])
                        nc.sync.dma_start(otdump[:, :], ot32[1:65, :])
                    j0 = (qt % 2) * 4
                    nc.sync.dma_start(
                        a2a_in[(b, qt // 2)][
                            j0 : j0 + 4, h * 64 : (h + 1) * 64, :
                        ].rearrange("j p e -> p j e"),
                        o_t[1:65, :].rearrange("p (j e) -> p j e", j=4),
                    )
                if qt % 2 == 1:
                    emit_a2a(b, qt // 2)

            for b in range(B):
                # filler inventory for this batch's spine
                if b == 0:
                    for t in (1, 2, 3):
                        fillers.append(gen_qkv_sec(t, "k"))
                        fillers.append(gen_qkv_sec(t, "v"))
                    for t in (1, 2, 3):
                        fillers.append(gen_qkv_sec(t, "q"))
                    for t in range(4, 8):
                        for sec in ("k", "v", "q"):
                            fillers.append(gen_qkv_sec(t, sec))
                    # outproj(0,0) deferred to b1: the first A2A is slow
                elif b == 1:
                    for sec in ("k", "v", "q"):
                        fillers.append(gen_qkv_sec(8, sec))
                    fillers.append(gen_outproj(0, 0))
                    fillers.append(gen_outproj(0, 1))
                    for sec in ("k", "v", "q"):
                        fillers.append(gen_qkv_sec(9, sec))
                    for sec in ("k", "v", "q"):
                        fillers.append(gen_qkv_sec(10, sec))
                    fillers.append(gen_outproj(1, 0))
                    for sec in ("k", "v", "q"):
                        fillers.append(gen_qkv_sec(11, sec))
                elif b == 2:
                    for sec in ("k", "v", "q"):
                        fillers.append(gen_qkv_sec(12, sec))
                    fillers.append(gen_outproj(1, 1))
                    for sec in ("k", "v", "q"):
                        fillers.append(gen_qkv_sec(13, sec))
                    for sec in ("k", "v", "q"):
                        fillers.append(gen_qkv_sec(14, sec))
                    fillers.append(gen_outproj(2, 0))
                    for sec in ("k", "v", "q"):
                        fillers.append(gen_qkv_sec(15, sec))
                else:
                    fillers.append(gen_outproj(b - 1, 1))
                    # gen_outproj(b, 0) is appended mid-spine, after its A2A

                if b == 0:
                    quotas = (2304, 1024, 1024, 1024)
                elif b == 1:
                    quotas = (1152, 1152, 1152, 1152)
                elif b == 2:
                    quotas = (1024, 1024, 1024, 1024)
                else:
                    quotas = (320, 320, 320, 320)
                for qt in range(4):
                    quota0 = quotas[qt]
                    # prefetch next batch's x tiles (consolidated DMAs)
                    if b < B - 1:
                        nt = 4 * (b + 1)
                        if qt == 0:
                            load_x_tile(nt + 0)
                            load_x_tile(nt + 1)
                        elif qt == 1:
                            load_x_tile(nt + 2)
                            load_x_tile(nt + 3)
                    pv = [
                        psum.tile([128, 512], FP32, tag="pv", name=f"pv{h}")
                        for h in range(HL)
                    ]
                    q_off = b * NTOK + qt * TN
                    for kc in range(KT):
                        s_t = psum.tile([128, 1024], FP32, tag="st", name="s_t")
                        for h in range(HL):
                            nc.tensor.matmul(
                                s_t[:, h * 512 : (h + 1) * 512],
                                lhsT=k_sb[
                                    h * 64 : (h + 1) * 64,
                                    b * NTOK + kc * 128 : b * NTOK + (kc + 1) * 128,
                                ],
                                rhs=q_sb[h * 64 : (h + 1) * 64, q_off : q_off + TN],
                                start=True,
                                stop=True,
                            )
                        p_t = probs.tile([128, 1024], BF16, tag="p", name="p_t")
                        nc.scalar.activation(p_t[:], s_t[:], AF.Exp)
                        pending.append((b * KT + kc, p_t, pv))
                        if kc == 2 and fin_q:
                            finish_qt(*fin_q.pop(0))
                            if b == B - 1 and qt == 2:
                                # A2A(3,0) just emitted; safe to schedule its
                                # outproj as filler from here on
                                fillers.append(gen_outproj(b, 0))
                        # hold b3's filler until its A2A deps are in flight
                        if not (b == B - 1 and qt == 0 and kc < 3):
                            pump(quota0)
                        while len(pending) > 2:
                            flush_one()
                    fin_q.append((b, qt, pv))

            # ---- tail ----
            while pending:
                flush_one()
            exhaust(-1)  # any leftover filler (incl. outproj(3,0) remainder)
            finish_qt(*fin_q.pop(0))  # qt3 of batch 3 -> emits A2A(3,1)
            # p-state keepalive: harmless matmuls keep the PE clocked up
            # while the final A2A + gather complete, so the last outproj
            # runs at full rate instead of the cold ~2x-slow rate
            dum = psum.tile([128, 512], FP32, tag="fil", name="dum")
            for _ in range(56):
                nc.tensor.matmul(
                    dum[:, :], lhsT=w_sb[:, 0:128], rhs=wo_sb[:, 0:512],
                    start=True, stop=True,
                )
            for _ in gen_outproj(B - 1, 1):
                pass

            if debug:
                for t in range(T // TN):
                    d1 = fin.tile([128, TN], FP32, tag="outt", name="d1")
                    nc.vector.tensor_copy(d1[:], q_sb[:, t * TN : (t + 1) * TN])
                    nc.sync.dma_start(qdump[:, t * TN : (t + 1) * TN], d1[:])
                    d2 = fin.tile([128, TN], FP32, tag="outt", name="d2")
                    nc.vector.tensor_copy(d2[:], k_sb[:, t * TN : (t + 1) * TN])
                    nc.sync.dma_start(kdump[:, t * TN : (t + 1) * TN], d2[:])
                nv = (T // 128) * HL * 65
                for j in range(0, nv, 1024):
                    wdt = min(1024, nv - j)
                    d3 = fin.tile([128, 1024], FP32, tag="outt", name="d3")
                    nc.vector.tensor_copy(d3[:, 0:wdt], v_sb[:, j : j + wdt])
                    nc.sync.dma_start(vdump[:, j : j + wdt], d3[:, 0:wdt])
                for i in range(N_CORES):
                    d4 = fin.tile([128, 128], BF16, tag="d4", name="d4")
                    nc.sync.dma_start(d4[:], a2a_in[(0, 0)][i, :, :])
                    d5 = fin.tile([128, 128], FP32, tag="outt", name="d5")
                    nc.vector.tensor_copy(d5[:], d4[:])
                    nc.sync.dma_start(adump[i, :, :], d5[:])

    nc.compile()
    return nc


_NC_CACHE = None


def _get_nc():
    global _NC_CACHE
    if _NC_CACHE is None:
        _NC_CACHE = build_nc()
    return _NC_CACHE


def make_in_maps(x, w_qkv, w_out, b_out):
    x = np.asarray(x, dtype=np.float32)
    w_qkv = np.asarray(w_qkv, dtype=np.float32)
    w_out = np.asarray(w_out, dtype=np.float32)
    b_out = np.asarray(b_out, dtype=np.float32)

    xt_np = np.ascontiguousarray(x.reshape(T, D).T).astype(ml_dtypes.bfloat16)
    wo_np = np.ascontiguousarray(w_out.T).astype(ml_dtypes.bfloat16)
    b_np = np.ascontiguousarray(b_out.reshape(1, D))

    in_maps = []
    for c in range(N_CORES):
        rows = []
        for sec in range(3):  # q, k, v sections of w_qkv
            for hh in range(HL):
                h = HL * c + hh
                rows.append(w_qkv[sec * D + h * HD : sec * D + (h + 1) * HD, :])
        wt_np = np.ascontiguousarray(np.concatenate(rows, 0).T).astype(
            ml_dtypes.bfloat16
        )  # (1024, 384)
        in_maps.append({"xt": xt_np, "wt": wt_np, "wo": wo_np, "bias": b_np})
    return in_maps


def kernel(x, w_qkv, w_out, b_out, _trace=False, _tmpdir=None):
    in_maps = make_in_maps(x, w_qkv, w_out, b_out)
    nc = _get_nc()
    res = bass_utils.run_bass_kernel_spmd(
        nc, in_maps, core_ids=list(range(N_CORES)), trace=_trace, tmpdir=_tmpdir
    )
    # core j out rows: (b, hf) block r = (2b+hf)*128 + u
    #   -> token b*2048 + hf*1024 + j*128 + u of the full output
    full = np.empty((T, D), np.float32)
    for j in range(N_CORES):
        o = np.asarray(res.results[j]["out"], dtype=np.float32)
        for b in range(B):
            for hf in range(2):
                dst = b * NTOK + hf * 1024 + j * 128
                src = (b * 2 + hf) * 128
                full[dst : dst + 128] = o[src : src + 128]
    kernel.last_result = res
    return full.reshape(B, NTOK, D)
